# revision 33
# baseline (speedup 1.0000x reference)
"""Trainium2 Bass kernel for pre-LN multi-head attention block.

Reference computation (per batch element):
  xn = LayerNorm(x) * gamma + beta                 [N, D]
  qkv = xn @ w_qkv.T                               [N, 3*INNER]
  q, k, v -> [H, N, Dh]; attn = softmax(q k^T / sqrt(Dh)); o = attn @ v
  out = o @ w_proj.T + b_proj                      [N, D]

Sharding: data-parallel over batch B=8 across the 8 NeuronCores (one batch
element per core, no collectives).

The softmax exp is the scalar-engine bottleneck (33.5M exps/core), so it is
split between the Scalar engine (exact exp -> bf16) and the Vector engine
(Schraudolph bit-trick: int16(x*A+B) == bf16 bits of ~exp(x), rms 1.8%).
Score tiles triple-buffer in PSUM so both exp engines stream without stalls.

Fast path requires w_qkv @ ln_beta == 0 (true when ln_beta == 0); gamma is
folded into the weights on the host. Otherwise a general fallback graph with
on-device biases is used.

Shapes (hardcoded): B=8, N=2048, D=512, H=8, Dh=64, INNER=512.
"""

import os
import numpy as np
import ml_dtypes

import concourse.bass as bass
import concourse.mybir as mybir
import concourse.tile as tile
from concourse import bacc, masks

F32 = mybir.dt.float32
BF16 = mybir.dt.bfloat16
I16 = mybir.dt.int16

B = 8
N = 2048
D = 512
H = 8
Dh = 64
INNER = H * Dh  # 512
EPS = 1e-6
SCALE = Dh ** -0.5  # 0.125

P = 128
NT = N // P       # 16 token tiles
DC = D // P       # 4 d-chunks
QT = 4            # q tiles of 512
QW = N // QT      # 512
KC = N // P       # 16 key chunks of 128
HT = H // 2       # 4 head pairs (2 heads share a 128-partition tile)

LN2 = float(np.log(2.0))
A16 = SCALE * 128.0 / LN2         # Schraudolph slope (bf16 bits per score unit)
B16 = 16256.0 - 7.0               # offset; calibrated for trunc-convert, zero mean bias

# exp engine assignment per (t,s) block: one entry per key chunk. Engines
# MUST alternate chunk-to-chunk: the score-PSUM ring (2 tiles) recycles at
# the rate of the engine that ran exp(k-2), so same-engine runs serialize it.
# 0 = scalar engine (exact exp), 1 = vector engine (Schraudolph approx)
PAT_EVEN = [0, 1] * 8                 # strict alternation
PAT_ODD = [0, 1] * 8


def build_graph_fast():
    nc = bacc.Bacc()

    x = nc.declare_dram_parameter("x", [N, D], F32, isOutput=False)
    w_qkvT = nc.declare_dram_parameter("w_qkvT", [D, 3 * INNER], BF16, isOutput=False)
    w_projT = nc.declare_dram_parameter("w_projT", [INNER, D], BF16, isOutput=False)
    b_proj = nc.declare_dram_parameter("b_proj", [D], F32, isOutput=False)
    out = nc.declare_dram_parameter("out", [N, D], F32, isOutput=True)

    with tile.TileContext(nc) as tc:
        with (
            tc.tile_pool(name="consts", bufs=1) as consts,
            tc.tile_pool(name="big", bufs=1) as big,
            tc.tile_pool(name="ln", bufs=4) as ln,
            tc.tile_pool(name="xload", bufs=6) as xload,
            tc.tile_pool(name="yout", bufs=4) as yout,
            tc.tile_pool(name="etp", bufs=14) as etp,
            tc.tile_pool(name="small", bufs=4) as small,
            tc.tile_pool(name="s_ps", bufs=2, space="PSUM") as s_ps,
            tc.tile_pool(name="o_ps", bufs=1, space="PSUM") as o_ps,
        ):
            # PSUM: s_ps "ps" = [128,1024] f32 x2 bufs (4 banks), exclusive
            # to attention score tiles so scores are never gated behind other
            # phases; o_ps po_a{0,1}/po_b{0,1} 1 bank each for P@V
            # accumulators, borrowed round-robin by QKV/transpose/proj.
            _mm_ctr = [0]
            _mm_tags = ["po_a0", "po_b0", "po_a1", "po_b1"]

            def mm_ps_tile(shape, dtype):
                _mm_ctr[0] += 1
                return o_ps.tile(shape, dtype,
                                 tag=_mm_tags[_mm_ctr[0] % 4],
                                 name=f"mm_{_mm_ctr[0]}")

            # ---- constants ----
            wq = consts.tile([P, DC, 3 * INNER], BF16)
            nc.gpsimd.dma_start(wq, w_qkvT.rearrange("(o p) f -> p o f", p=P))
            wp = consts.tile([P, DC, D], BF16)
            nc.gpsimd.dma_start(wp, w_projT.rearrange("(o p) f -> p o f", p=P))
            beff_row = consts.tile([1, D], BF16)
            nc.gpsimd.dma_start(beff_row, b_proj.rearrange("(a d) -> a d", a=1))
            ones_col = consts.tile([1, P], BF16)
            nc.vector.memset(ones_col, 1.0)
            eps_t = consts.tile([P, 1], F32)
            nc.vector.memset(eps_t, EPS)
            b16_t = consts.tile([P, 1], F32)
            nc.vector.memset(b16_t, B16)
            ident = consts.tile([P, P], BF16)
            masks.make_identity(nc, ident)

            # ---- LayerNorm -> transpose -> QKV, pipelined per s-chunk ----
            # (gamma is folded into w_qkv on the host)
            xnT = [big.tile([P, DC, QW], BF16, name=f"xnT{s}") for s in range(QT)]
            qT = big.tile([P, HT, N], BF16)
            kT = big.tile([P, HT, N], BF16)
            v_aug = big.tile([P, KC, H, Dh + 1], BF16)
            nc.vector.memset(v_aug[:, :, :, Dh:Dh + 1], 1.0)

            # V pieces are queued and drained one-per-chunk inside the
            # attention stream: they fill the PE while early attention waits
            # for the remaining K chunks, instead of delaying attention start.
            pending_v = []

            def v_piece(nt):
                s_, j = nt // 4, nt % 4
                ps = s_ps.tile([P, INNER], F32, tag="ps", name=f"vp{nt}")
                for dc in range(DC):
                    nc.tensor.matmul(ps,
                                     lhsT=xnT[s_][:, dc, j * P:(j + 1) * P],
                                     rhs=wq[:, dc, 2 * INNER:3 * INNER],
                                     start=(dc == 0), stop=(dc == DC - 1))
                nc.scalar.copy(
                    v_aug[:, nt, :, 0:Dh],
                    ps[:, :].rearrange("p (h c) -> p h c", h=H))

            def q_piece(sf):
                s_, f = sf // HT, sf % HT
                ps = s_ps.tile([P, QW], F32, tag="ps", name=f"qp{sf}")
                for dc in range(DC):
                    nc.tensor.matmul(ps,
                                     lhsT=wq[:, dc, f * P:(f + 1) * P],
                                     rhs=xnT[s_][:, dc, :],
                                     start=(dc == 0), stop=(dc == DC - 1))
                if f % 2 == 0:
                    nc.vector.tensor_copy(qT[:, f, s_ * QW:(s_ + 1) * QW], ps)
                else:
                    nc.scalar.copy(qT[:, f, s_ * QW:(s_ + 1) * QW], ps)

            for nt in range(NT):
                pending_v.append(("v", nt))

            # ---- attention (s outer so proj drains per s-chunk) ----
            oT = [big.tile([P, HT, QW], BF16, name=f"oT{s}") for s in range(QT)]

            # Global software pipeline: every chunk's PV pair is queued and
            # emitted PV_LAG chunk-slots later (spilling into the next
            # block), so the PE instruction queue never has a PV that is
            # about to wait on a fresh exp in front of ready score matmuls.
            PV_LAG = 5
            pending_pv = []

            def drain_pv():
                if pending_pv:
                    pending_pv.pop(0)()

            def attn_chunks(t, s):
                blk = t * QT + s
                pat = PAT_EVEN if blk % 2 == 0 else PAT_ODD
                par = t % 2
                po_a = o_ps.tile([P, QW], F32, tag=f"po_a{par}",
                                 name=f"poa{t}{s}")
                po_b = o_ps.tile([P, QW], F32, tag=f"po_b{par}",
                                 name=f"pob{t}{s}")
                ets = {}

                def pv_piece(kc, _t=t, _po_a=po_a, _po_b=po_b):
                    et = ets.pop(kc)
                    nc.tensor.matmul(_po_a[0:Dh + 1, :],
                                     lhsT=v_aug[:, kc, 2 * _t, :],
                                     rhs=et[:, 0:QW],
                                     start=(kc == 0), stop=(kc == KC - 1))
                    nc.tensor.matmul(_po_b[0:Dh + 1, :],
                                     lhsT=v_aug[:, kc, 2 * _t + 1, :],
                                     rhs=et[:, QW:2 * QW],
                                     start=(kc == 0), stop=(kc == KC - 1))

                for kc in range(KC):
                    ps = s_ps.tile([P, 2 * QW], F32, tag="ps",
                                   name=f"s{t}{s}{kc}")
                    nc.tensor.matmul(
                        ps[:, 0:QW],
                        lhsT=kT[0:Dh, t, kc * P:(kc + 1) * P],
                        rhs=qT[0:Dh, t, s * QW:(s + 1) * QW],
                        start=True, stop=True)
                    nc.tensor.matmul(
                        ps[:, QW:2 * QW],
                        lhsT=kT[Dh:P, t, kc * P:(kc + 1) * P],
                        rhs=qT[Dh:P, t, s * QW:(s + 1) * QW],
                        start=True, stop=True)
                    et = etp.tile([P, 2 * QW], BF16, tag="et",
                                  name=f"et{t}{s}{kc}")
                    ets[kc] = et
                    if pat[kc] == 0:
                        nc.scalar.activation(et, ps,
                                             mybir.ActivationFunctionType.Exp,
                                             scale=SCALE)
                    else:
                        nc.vector.scalar_tensor_tensor(
                            out=et.bitcast(I16), in0=ps, scalar=A16,
                            in1=b16_t.broadcast_to((P, 2 * QW)),
                            op0=mybir.AluOpType.mult,
                            op1=mybir.AluOpType.add)
                    if pending_v:
                        kind, idx = pending_v.pop(0)
                        (v_piece if kind == "v" else q_piece)(idx)
                    pending_pv.append(
                        (lambda _kc=kc, _f=pv_piece: _f(_kc)))
                    if len(pending_pv) > PV_LAG:
                        drain_pv()
                    yield
                # normalize: O = O~ / rowsum (rowsum in row 64).
                # reciprocal_approx_fast requires SBUF input (PSUM src is
                # broken on HW), so stage the rowsum rows via scalar copies.
                # Normalize tail (queued; emitted right after this block's
                # last PV drains from the global PV pipeline).
                def norm_tail(_t=t, _s=s, _po_a=po_a, _po_b=po_b):
                    for h_off, po in ((0, _po_a), (1, _po_b)):
                        rs = small.tile([1, QW], F32, tag="rs",
                                        name=f"rs{_t}{_s}{h_off}")
                        nc.scalar.copy(rs, po[Dh:Dh + 1, :])
                        rr = small.tile([1, QW], F32, tag="rr",
                                        name=f"rr{_t}{_s}{h_off}")
                        nc.vector.reciprocal_approx_fast(out=rr, in_=rs)
                        rb = small.tile([Dh, QW], F32, tag="rb",
                                        name=f"rb{_t}{_s}{h_off}")
                        nc.gpsimd.partition_broadcast(rb, rr)
                        nc.vector.tensor_tensor(
                            oT[_s][h_off * Dh:(h_off + 1) * Dh, _t, :],
                            po[0:Dh, :], rb, mybir.AluOpType.mult)

                pending_pv.append(norm_tail)
                yield

            def attn_block(t, s):
                for _ in attn_chunks(t, s):
                    pass

            def proj_piece(s, j):
                # Borrows a po tag whose attention-block holder has just
                # released it (blocks alternate parity by t), leaving the
                # score-PSUM ring alone.
                nt = s * 4 + j
                ps = o_ps.tile([P, D], F32,
                               tag=("po_a" if j % 2 == 0 else "po_b")
                               + str(j % 2),
                               name=f"prj{nt}")
                nc.tensor.matmul(ps, lhsT=ones_col, rhs=beff_row,
                                 start=True, stop=False)
                for c in range(DC):
                    nc.tensor.matmul(ps,
                                     lhsT=oT[s][:, c, j * P:(j + 1) * P],
                                     rhs=wp[:, c, :],
                                     start=False, stop=(c == DC - 1))
                yt = yout.tile([P, D], F32, tag="yt", name=f"yt_{nt}")
                nc.scalar.copy(yt, ps)
                nc.sync.dma_start(out[nt * P:(nt + 1) * P, :], yt)

            for s in range(QT):
                for i in range(4 * s, 4 * s + 4):
                    xt = xload.tile([P, D], F32, tag="xt", name=f"xt{i}")
                    nc.sync.dma_start(xt, x[i * P:(i + 1) * P, :])
                    stats = ln.tile([P, 6], F32, tag="stats", name=f"st{i}")
                    nc.vector.bn_stats(stats, xt)
                    mv = ln.tile([P, 2], F32, tag="mv", name=f"mv{i}")
                    nc.vector.bn_aggr(mv, stats)
                    std = ln.tile([P, 1], F32, tag="std", name=f"sd{i}")
                    nc.scalar.activation(std, mv[:, 1:2],
                                         mybir.ActivationFunctionType.Sqrt,
                                         bias=eps_t)
                    rstd = ln.tile([P, 1], F32, tag="rstd", name=f"rd{i}")
                    nc.vector.reciprocal(rstd, std)
                    xn_b = ln.tile([P, D], BF16, tag="xn_b", name=f"xb{i}")
                    nc.vector.tensor_scalar(out=xn_b, in0=xt,
                                            scalar1=mv[:, 0:1], scalar2=rstd,
                                            op0=mybir.AluOpType.subtract,
                                            op1=mybir.AluOpType.mult)
                    for dc in range(DC):
                        pt = mm_ps_tile([P, P], BF16)
                        nc.tensor.transpose(pt, xn_b[:, dc * P:(dc + 1) * P],
                                            ident)
                        dst = xnT[s][:, dc, (i % 4) * P:(i % 4 + 1) * P]
                        nc.scalar.copy(dst, pt)

                # K for every s-chunk up front (attention scores need all
                # of K); Q only for s=0 — the later Q features trail into
                # the attention stream as fillers, like V.
                feats = (HT, HT + 1, HT + 2, HT + 3) + ((0, 1, 2, 3)
                                                        if s == 0 else ())
                for f in feats:
                    dest = qT if f < HT else kT
                    ft = f % HT
                    ps = mm_ps_tile([P, QW], F32)
                    for dc in range(DC):
                        nc.tensor.matmul(ps,
                                         lhsT=wq[:, dc, f * P:(f + 1) * P],
                                         rhs=xnT[s][:, dc, :],
                                         start=(dc == 0), stop=(dc == DC - 1))
                    if f % 2 == 0:
                        nc.vector.tensor_copy(dest[:, ft, s * QW:(s + 1) * QW],
                                              ps)
                    else:
                        nc.scalar.copy(dest[:, ft, s * QW:(s + 1) * QW], ps)
                if s >= 1:
                    for f in range(HT):
                        pending_v.append(("q", s * HT + f))
            for s in range(QT):
                for t in range(HT):
                    attn_block(t, s)
                    # proj for the previous s-chunk, one piece per block so
                    # the PE queue is never starved for long.
                    if s >= 1 and t >= 1:
                        proj_piece(s - 1, t - 1)
                if s >= 1:
                    proj_piece(s - 1, 3)
            while pending_pv:
                drain_pv()
            for j in range(4):
                proj_piece(QT - 1, j)

    nc.compile()
    return nc


def build_graph_general():
    # Slow-but-general fallback (biases applied on device); equivalent to the
    # previous baseline kernel. Only used when w_qkv @ ln_beta != 0.
    nc = bacc.Bacc()

    x = nc.declare_dram_parameter("x", [N, D], F32, isOutput=False)
    w_qkvT = nc.declare_dram_parameter("w_qkvT", [D, 3 * INNER], BF16, isOutput=False)
    b_qkv = nc.declare_dram_parameter("b_qkv", [3 * INNER], F32, isOutput=False)
    w_projT = nc.declare_dram_parameter("w_projT", [INNER, D], BF16, isOutput=False)
    b_proj = nc.declare_dram_parameter("b_proj", [D], F32, isOutput=False)
    out = nc.declare_dram_parameter("out", [N, D], F32, isOutput=True)

    def bcast_ap(ap_1d, parts):
        return bass.AP(tensor=ap_1d.tensor, offset=ap_1d.offset,
                       ap=[[0, parts]] + list(ap_1d.ap))

    with tile.TileContext(nc) as tc:
        with (
            tc.tile_pool(name="consts", bufs=1) as consts,
            tc.tile_pool(name="big", bufs=1) as big,
            tc.tile_pool(name="ln", bufs=4) as ln,
            tc.tile_pool(name="xload", bufs=6) as xload,
            tc.tile_pool(name="yout", bufs=4) as yout,
            tc.tile_pool(name="work", bufs=3) as work,
            tc.tile_pool(name="small", bufs=4) as small,
            tc.tile_pool(name="s_ps", bufs=2, space="PSUM") as s_ps,
            tc.tile_pool(name="o_ps", bufs=2, space="PSUM") as o_ps,
        ):
            _mm_ctr = [0]

            def mm_ps_tile(shape, dtype):
                _mm_ctr[0] += 1
                tag = "po_a" if _mm_ctr[0] % 2 else "po_b"
                return o_ps.tile(shape, dtype, tag=tag,
                                 name=f"mm_{_mm_ctr[0]}")

            wq = consts.tile([P, DC, 3 * INNER], BF16)
            nc.gpsimd.dma_start(wq, w_qkvT.rearrange("(o p) f -> p o f", p=P))
            wp = consts.tile([P, DC, D], BF16)
            nc.gpsimd.dma_start(wp, w_projT.rearrange("(o p) f -> p o f", p=P))
            bqkv_col = consts.tile([P, 3 * INNER // P], F32)
            nc.gpsimd.dma_start(bqkv_col, b_qkv.rearrange("(o p) -> p o", p=P))
            bv_bc = consts.tile([P, INNER], F32)
            nc.gpsimd.dma_start(bv_bc, bcast_ap(b_qkv[2 * INNER:3 * INNER], P))
            bias_bc = consts.tile([P, D], F32)
            nc.gpsimd.dma_start(bias_bc, bcast_ap(b_proj[:], P))
            eps_t = consts.tile([P, 1], F32)
            nc.vector.memset(eps_t, EPS)
            ident = consts.tile([P, P], BF16)
            masks.make_identity(nc, ident)

            xnT = [big.tile([P, DC, QW], BF16, name=f"xnT{s}") for s in range(QT)]
            qT = big.tile([P, HT, N], BF16)
            kT = big.tile([P, HT, N], BF16)
            v_aug = big.tile([P, KC, H, Dh + 1], BF16)
            nc.vector.memset(v_aug[:, :, :, Dh:Dh + 1], 1.0)

            for s in range(QT):
                for i in range(4 * s, 4 * s + 4):
                    xt = xload.tile([P, D], F32, tag="xt", name=f"xt{i}")
                    nc.sync.dma_start(xt, x[i * P:(i + 1) * P, :])
                    stats = ln.tile([P, 6], F32, tag="stats", name=f"st{i}")
                    nc.vector.bn_stats(stats, xt)
                    mv = ln.tile([P, 2], F32, tag="mv", name=f"mv{i}")
                    nc.vector.bn_aggr(mv, stats)
                    std = ln.tile([P, 1], F32, tag="std", name=f"sd{i}")
                    nc.scalar.activation(std, mv[:, 1:2],
                                         mybir.ActivationFunctionType.Sqrt,
                                         bias=eps_t)
                    rstd = ln.tile([P, 1], F32, tag="rstd", name=f"rd{i}")
                    nc.vector.reciprocal(rstd, std)
                    xn_b = ln.tile([P, D], BF16, tag="xn_b", name=f"xb{i}")
                    nc.vector.tensor_scalar(out=xn_b, in0=xt,
                                            scalar1=mv[:, 0:1], scalar2=rstd,
                                            op0=mybir.AluOpType.subtract,
                                            op1=mybir.AluOpType.mult)
                    for dc in range(DC):
                        pt = mm_ps_tile([P, P], BF16)
                        nc.tensor.transpose(pt, xn_b[:, dc * P:(dc + 1) * P],
                                            ident)
                        dst = xnT[s][:, dc, (i % 4) * P:(i % 4 + 1) * P]
                        nc.scalar.copy(dst, pt)

                for f in range(2 * HT):
                    dest = qT if f < HT else kT
                    ft = f % HT
                    ps = mm_ps_tile([P, QW], F32)
                    for dc in range(DC):
                        nc.tensor.matmul(ps,
                                         lhsT=wq[:, dc, f * P:(f + 1) * P],
                                         rhs=xnT[s][:, dc, :],
                                         start=(dc == 0), stop=(dc == DC - 1))
                    if f % 2 == 0:
                        nc.vector.tensor_scalar(
                            out=dest[:, ft, s * QW:(s + 1) * QW], in0=ps,
                            scalar1=bqkv_col[:, f:f + 1], scalar2=None,
                            op0=mybir.AluOpType.add)
                    else:
                        nc.scalar.activation(
                            dest[:, ft, s * QW:(s + 1) * QW], ps,
                            mybir.ActivationFunctionType.Identity,
                            bias=bqkv_col[:, f:f + 1])
                for j in range(4):
                    nt = s * 4 + j
                    ps = mm_ps_tile([P, INNER], F32)
                    for dc in range(DC):
                        nc.tensor.matmul(ps,
                                         lhsT=xnT[s][:, dc, j * P:(j + 1) * P],
                                         rhs=wq[:, dc, 2 * INNER:3 * INNER],
                                         start=(dc == 0), stop=(dc == DC - 1))
                    nc.vector.tensor_tensor(
                        v_aug[:, nt, :, 0:Dh],
                        ps[:, :].rearrange("p (h c) -> p h c", h=H),
                        bv_bc[:, :].rearrange("p (h c) -> p h c", h=H),
                        mybir.AluOpType.add)

            oT = [big.tile([P, DC, QW], BF16, name=f"oT{s}") for s in range(QT)]

            def proj_piece(s, j):
                # Borrows a po tag whose attention-block holder has just
                # released it (blocks alternate parity by t), leaving the
                # score-PSUM ring alone.
                nt = s * 4 + j
                ps = o_ps.tile([P, D], F32,
                               tag=("po_a" if j % 2 == 0 else "po_b")
                               + str(j % 2),
                               name=f"prj{nt}")
                for c in range(DC):
                    nc.tensor.matmul(ps,
                                     lhsT=oT[s][:, c, j * P:(j + 1) * P],
                                     rhs=wp[:, c, :],
                                     start=(c == 0), stop=(c == DC - 1))
                yt = yout.tile([P, D], F32, tag="yt", name=f"yt_{nt}")
                nc.vector.tensor_tensor(yt, ps, bias_bc, mybir.AluOpType.add)
                nc.sync.dma_start(out[nt * P:(nt + 1) * P, :], yt)

            for t in range(HT):
                for s in range(QT):
                    po_a = o_ps.tile([P, QW], F32, tag="po_a", name=f"poa{t}{s}")
                    po_b = o_ps.tile([P, QW], F32, tag="po_b", name=f"pob{t}{s}")
                    for kc in range(KC):
                        ps = s_ps.tile([P, 2 * QW], F32, tag="ps",
                                       name=f"s{t}{s}{kc}")
                        nc.tensor.matmul(
                            ps[:, 0:QW],
                            lhsT=kT[0:Dh, t, kc * P:(kc + 1) * P],
                            rhs=qT[0:Dh, t, s * QW:(s + 1) * QW],
                            start=True, stop=True)
                        nc.tensor.matmul(
                            ps[:, QW:2 * QW],
                            lhsT=kT[Dh:P, t, kc * P:(kc + 1) * P],
                            rhs=qT[Dh:P, t, s * QW:(s + 1) * QW],
                            start=True, stop=True)
                        et = work.tile([P, 2 * QW], BF16, tag="et",
                                       name=f"et{t}{s}{kc}")
                        nc.scalar.activation(et, ps,
                                             mybir.ActivationFunctionType.Exp,
                                             scale=SCALE)
                        nc.tensor.matmul(po_a[0:Dh + 1, :],
                                         lhsT=v_aug[:, kc, 2 * t, :],
                                         rhs=et[:, 0:QW],
                                         start=(kc == 0), stop=(kc == KC - 1))
                        nc.tensor.matmul(po_b[0:Dh + 1, :],
                                         lhsT=v_aug[:, kc, 2 * t + 1, :],
                                         rhs=et[:, QW:2 * QW],
                                         start=(kc == 0), stop=(kc == KC - 1))
                    for h_off, po in ((0, po_a), (1, po_b)):
                        rs = small.tile([1, QW], F32, tag="rs")
                        nc.vector.tensor_copy(rs, po[Dh:Dh + 1, :])
                        ot_tmp = small.tile([Dh, QW], BF16, tag="ot_tmp")
                        nc.vector.tensor_copy(ot_tmp, po[0:Dh, :])
                        rr = small.tile([1, QW], F32, tag="rr")
                        nc.vector.reciprocal_approx_fast(out=rr, in_=rs)
                        rb = small.tile([Dh, QW], F32, tag="rb")
                        nc.gpsimd.partition_broadcast(rb, rr)
                        nc.vector.tensor_tensor(
                            oT[s][h_off * Dh:(h_off + 1) * Dh, t, :],
                            ot_tmp, rb, mybir.AluOpType.mult)
            for s in range(QT):
                for j in range(4):
                    proj_piece(s, j)

    nc.compile()
    return nc


_CACHED = {}


def _prep_weights(w_qkv, w_proj, b_proj, ln_gamma, ln_beta):
    # Fold LN affine into the QKV projection:
    #   (xn * gamma + beta) @ W^T == xn @ (W * gamma)^T + beta @ W^T
    w_qkv = np.asarray(w_qkv, dtype=np.float64)
    w_proj = np.asarray(w_proj, dtype=np.float64)
    gamma = np.asarray(ln_gamma, dtype=np.float64)
    beta = np.asarray(ln_beta, dtype=np.float64)
    w_eff = w_qkv * gamma[None, :]
    b_qkv = w_qkv @ beta
    fast = bool(np.max(np.abs(b_qkv)) == 0.0)
    shared = {
        "w_qkvT": np.ascontiguousarray(w_eff.T).astype(ml_dtypes.bfloat16),
        "w_projT": np.ascontiguousarray(w_proj.T).astype(ml_dtypes.bfloat16),
        "b_proj": np.ascontiguousarray(b_proj).astype(np.float32),
    }
    if not fast:
        shared["b_qkv"] = np.ascontiguousarray(b_qkv).astype(np.float32)
    return fast, shared


def kernel(x, w_qkv, w_proj, b_proj, ln_gamma, ln_beta):
    from concourse.bass_utils import run_bass_kernel_spmd

    x = np.asarray(x, dtype=np.float32)
    assert x.shape == (B, N, D), x.shape

    fast, shared = _prep_weights(np.asarray(w_qkv), np.asarray(w_proj),
                                 np.asarray(b_proj), np.asarray(ln_gamma),
                                 np.asarray(ln_beta))

    key = "nc_fast" if fast else "nc_gen"
    if key not in _CACHED:
        _CACHED[key] = build_graph_fast() if fast else build_graph_general()
    nc = _CACHED[key]

    in_maps = [dict(shared, x=np.ascontiguousarray(x[i])) for i in range(B)]

    trace = bool(int(os.environ.get("KERNEL_TRACE", "0")))
    res = run_bass_kernel_spmd(nc, in_maps, core_ids=list(range(B)),
                               trace=trace)
    if trace:
        _CACHED["exec_time_ns"] = res.exec_time_ns
        _CACHED["last_result"] = res
    outs = [np.asarray(res.results[i]["out"], dtype=np.float32)
            for i in range(B)]
    return np.stack(outs, axis=0)
